# revision 1
# baseline (speedup 1.0000x reference)
"""CRF negative-log-likelihood loss kernel for Trainium2 (8 NeuronCores).

Strategy (data-parallel over batch, 32 batch rows per core):

Denominator (forward algorithm) in LINEAR space, meet-in-the-middle:
    logsumexp_i(alpha_i + trans_ij) == log((exp(alpha) @ exp(trans))_j)
so with E = exp(trans), A_t = exp(em_t - c0) the forward state
u_t = (E^T u_{t-1}) * A_t and the backward state
v_{t-1} = E (A_t * v_t) + expend*d_{t-1}  (d_t[b] = [t == len(b)-1])
meet at t* = 255 (all lengths >= 256), where
    denom_b = log(sum_i u_255[i,b] * v_255[i,b]) + len(b)*c0.
Each chain is one PE matmul + one DVE elementwise multiply per step;
the two chains are independent, so their serial latencies overlap and
the wall time is ~half of a single 511-step chain.  Variable lengths
cost nothing in the loop: backward emissions are mask-zeroed so dead
batches carry v=0 until a rank-1 PSUM-accumulated inject matmul
(expend outer d_t) plants exp(end_transitions) at each batch's own
endpoint.  c0 is a constant per-step rescale that keeps everything in
fp32 range, accounted exactly on the host as len(b)*c0.

Numerator (gold path score):
  - emission gathers: one-hot (iota == label) * emission fused into one
    scalar_tensor_tensor with accumulate per (batch, s-chunk) tile; the
    mask is folded in on the host by pointing masked labels out of range.
  - transition gathers (+ start/end transitions): indirect_copy from a
    flattened transition table (quartered to fit SBUF) with a zero slot
    for masked steps; indices are host-prepared.

Everything is reduced by a final ones-vector matmul into a [1,168]
output per core; the host does the final logs / mean in float64.
"""

import numpy as np
from contextlib import ExitStack

B, S, T = 256, 512, 128
NCORES = 8
BC = B // NCORES          # batch rows per core
NCH = S // T              # 4 time chunks of 128 steps
MID = 255                 # meeting point t*; requires all len >= MID+1
C0 = float(np.log(211.0))  # per-step rescale in log space

# transition table layout (per quarter q of l_prev rows):
#   [0:4096)        trans[32q:32q+32, :] flattened
#   4096            0.0 (zero slot for masked / padding indices)
#   [4097:4225)     start_transitions
#   [4225:4353)     end_transitions
QT = 4368  # padded quarter-table length
NQ = 4


def _build_program(nv_per_group, need_mask, inj_rounds):
    """Build the SPMD Bass program (identical on all 8 cores).

    nv_per_group: num_valid_indices for each of the NQ quarter gathers
    (shared across cores; lists are padded to these sizes).
    need_mask: apply the mask multiply to backward-half emissions
    (False when the mask is all ones).
    inj_rounds: set of t values in [MID, S-1) where some batch ends, i.e.
    rounds whose inject outer-product matmul is actually nonzero.
    """
    import concourse.bacc as bacc
    import concourse.tile as tile
    import concourse.mybir as mybir
    from concourse.masks import make_identity

    f32 = mybir.dt.float32
    bf16 = mybir.dt.bfloat16
    u16 = mybir.dt.uint16
    ND = S - MID  # inject rows, t = MID .. S-1

    nc = bacc.Bacc()

    lg = nc.dram_tensor("lg", [BC, S, T], f32, kind="ExternalInput")
    labf = nc.dram_tensor("labf", [T, NCH, BC], f32, kind="ExternalInput")
    maskt = nc.dram_tensor("maskt", [T, 2, BC], f32, kind="ExternalInput")
    dmat = nc.dram_tensor("dmat", [1, ND, BC], bf16, kind="ExternalInput")
    transm = nc.dram_tensor("transm", [T, T], f32, kind="ExternalInput")
    startv = nc.dram_tensor("startv", [T, 1], f32, kind="ExternalInput")
    endr = nc.dram_tensor("endr", [1, T], f32, kind="ExternalInput")
    ttabs = nc.dram_tensor("ttabs", [NQ, QT], f32, kind="ExternalInput")
    gidx = nc.dram_tensor(
        "gidx", [NQ, 128, max(1, max(nv_per_group) // 16)], u16, kind="ExternalInput"
    )
    outv = nc.dram_tensor("outv", [1, 168], f32, kind="ExternalOutput")

    with tile.TileContext(nc) as tc, ExitStack() as ctx:
        consts = ctx.enter_context(tc.tile_pool(name="consts", bufs=1))
        abuf = ctx.enter_context(tc.tile_pool(name="abuf", bufs=1))
        stg = ctx.enter_context(tc.tile_pool(name="stg", bufs=4))
        osc = ctx.enter_context(tc.tile_pool(name="osc", bufs=2))
        uvp = ctx.enter_context(tc.tile_pool(name="uvp", bufs=2))
        ttp = ctx.enter_context(tc.tile_pool(name="ttp", bufs=1))
        gscr = ctx.enter_context(tc.tile_pool(name="gscr", bufs=2))
        qpool = ctx.enter_context(tc.tile_pool(name="qp", bufs=2, space="PSUM"))
        rpool = ctx.enter_context(tc.tile_pool(name="rp", bufs=2, space="PSUM"))
        tpool = ctx.enter_context(tc.tile_pool(name="tp", bufs=2, space="PSUM"))
        opool = ctx.enter_context(tc.tile_pool(name="op", bufs=1, space="PSUM"))

        # ---------------- constants ----------------
        ident = consts.tile([128, 128], f32)
        make_identity(nc, ident)

        tr_sb = consts.tile([T, T], f32)
        nc.sync.dma_start(tr_sb, transm[:, :])
        e_sb = consts.tile([T, T], bf16)
        nc.scalar.activation(e_sb, tr_sb, mybir.ActivationFunctionType.Exp)

        stv = consts.tile([T, 1], f32)
        nc.sync.dma_start(stv, startv[:, :])
        expstart = consts.tile([T, 1], f32)
        nc.scalar.activation(expstart, stv, mybir.ActivationFunctionType.Exp)

        enr = consts.tile([1, T], f32)
        nc.sync.dma_start(enr, endr[:, :])
        expendr = consts.tile([1, T], bf16)
        nc.scalar.activation(expendr, enr, mybir.ActivationFunctionType.Exp)

        ones = consts.tile([T, 1], f32)
        nc.vector.memset(ones, 1.0)

        minus_c0 = consts.tile([T, 1], f32)
        nc.vector.memset(minus_c0, -C0)

        iota = consts.tile([128, 128], f32)
        nc.gpsimd.iota(
            iota,
            pattern=[[1, 128]],
            base=0,
            channel_multiplier=0,
            allow_small_or_imprecise_dtypes=True,
        )

        lab_sb = consts.tile([T, NCH, BC], f32)
        nc.sync.dma_start(lab_sb, labf[:, :, :])

        msk_sb = consts.tile([T, 2, BC], f32)
        if need_mask:
            nc.sync.dma_start(msk_sb, maskt[:, :, :])

        d_sb = consts.tile([1, ND, BC], bf16)
        nc.sync.dma_start(d_sb, dmat[:, :, :])

        finalrhs = consts.tile([128, 168], f32)
        nc.vector.memset(finalrhs, 0.0)

        # ---------------- warmups ----------------
        # Each engine's first contact with another proc's output costs one
        # sync-wait slot; HW instruction structs allow only one wait, so
        # absorb first contacts with tiny ops (one new producer each).
        wd1 = consts.tile([128, 1], f32)
        wd2 = consts.tile([128, 1], f32)
        wd3 = consts.tile([1, 1], f32)
        wa = consts.tile([128, 1], f32)
        wg1 = consts.tile([128, 1], f32)
        wg2 = consts.tile([128, 1], f32)
        # DVE observes Pool (iota) then the lab/d DMA queues
        nc.vector.tensor_copy(wd1, iota[:, 0:1])
        nc.vector.tensor_copy(wd2, lab_sb[:, 0, 0:1])
        nc.vector.tensor_copy(wd3, d_sb[:, 0, 0:1])
        # ACT observes DVE (minus_c0 memset)
        nc.scalar.activation(wa, minus_c0, mybir.ActivationFunctionType.Exp)
        # gpsimd observes the lab DMA queue and ACT
        nc.gpsimd.tensor_copy(wg1, lab_sb[:, 0, 0:1])
        nc.gpsimd.tensor_copy(wg2, expstart)
        # PE observes Pool (identity) via a dummy transpose
        wpsum = tpool.tile([128, 128], f32, tag="tp")
        nc.tensor.transpose(wpsum, ident, ident)

        # E^T for the backward chain: transpose raw trans, then exp -> bf16
        etp = tpool.tile([128, 128], f32, tag="tp")
        nc.tensor.transpose(etp, tr_sb, ident)
        et_sb = consts.tile([T, T], bf16)
        nc.scalar.activation(et_sb, etp, mybir.ActivationFunctionType.Exp)

        # A chunks: [tags, t_local, batch]
        a_ch = [
            abuf.tile([T, T, BC], f32, tag=f"a{c}", name=f"a{c}") for c in range(NCH)
        ]

        stg_tiles = {}

        def emit_stage_dma(c):
            # split across 4 DMAs so they spread over HWDGE queues and the
            # first transpose group only waits for its own quarter
            st = stg.tile([T, BC, T], f32, tag="stg")
            src = lg[:, c * T:(c + 1) * T, :].rearrange("b s j -> s b j")
            for b0 in range(0, BC, 8):
                nc.sync.dma_start(st[:, b0 : b0 + 8, :], src[:, b0 : b0 + 8, :])
            stg_tiles[c] = st

        GB = 4  # batch rows per grouped transpose/exp

        def emit_gather_b(c, b):
            # fused one-hot emission gather (uses raw emissions):
            # out = (iota == label) * em ; accum -> finalrhs column
            st = stg_tiles[c]
            o = osc.tile([128, 128], f32, tag="osc")
            nc.vector.scalar_tensor_tensor(
                out=o,
                in0=iota,
                scalar=lab_sb[:, c, b : b + 1],
                in1=st[:, b, :],
                op0=mybir.AluOpType.is_equal,
                op1=mybir.AluOpType.mult,
                accum_out=finalrhs[:, c * BC + b : c * BC + b + 1],
            )

        def emit_pre_group(c, b0):
            # mask (log-space) for backward-half chunks, then 4 transposes
            # into one PSUM tile, one wide exp into the A chunk.
            st = stg_tiles[c]
            if need_mask and c >= 2:
                for b in range(b0, b0 + GB):
                    nc.vector.tensor_scalar(
                        out=st[:, b, :],
                        in0=st[:, b, :],
                        scalar1=msk_sb[:, c - 2, b : b + 1],
                        scalar2=None,
                        op0=mybir.AluOpType.add,
                    )
            tp = tpool.tile([128, GB, 128], f32, tag="tp")
            for i in range(GB):
                nc.tensor.transpose(tp[:, i, :], st[:, b0 + i, :], ident)
            # out[j, b, t] over GB batches matches psum free order (b, t)
            nc.scalar.activation(
                a_ch[c].rearrange("j t b -> j b t")[:, b0 : b0 + GB, :],
                tp,
                mybir.ActivationFunctionType.Exp,
                bias=minus_c0,
            )

        # ---------------- preprocessing chunks 0 and 3 ----------------
        emit_stage_dma(0)
        emit_stage_dma(3)
        for b0 in range(0, BC, GB):
            emit_pre_group(3, b0)
        for b0 in range(0, BC, GB):
            emit_pre_group(0, b0)
        emit_stage_dma(1)
        emit_stage_dma(2)

        # u_0 = exp(start) * A_0[:, 0, :]
        u_prev = uvp.tile([T, BC], bf16, tag="u", name="u_init")
        nc.vector.tensor_scalar(
            out=u_prev,
            in0=a_ch[0][:, 0, :],
            scalar1=expstart,
            scalar2=None,
            op0=mybir.AluOpType.mult,
        )

        # v_{S-1} = expend (x) d_{S-1}  (rank-1 outer product into PSUM)
        v_psum = rpool.tile([T, BC], f32, tag="r")
        nc.tensor.matmul(
            v_psum, expendr, d_sb[:, S - 1 - MID, :], start=True, stop=True
        )

        # transition-table quarter gathers, emitted piecewise inside the
        # round loop so the DVE reductions land in chain gaps, not the tail
        def emit_quarter_gather(qi):
            tt = ttp.tile([128, QT], f32, tag="ttab")
            nc.sync.dma_start(tt, ttabs[qi, :].partition_broadcast(128))
            gi = gscr.tile([128, max(nv_per_group) // 16], u16, tag="gi")
            nc.sync.dma_start(gi, gidx[qi, :, :])
            gout = gscr.tile([128, nv_per_group[qi]], f32, tag="gout")
            nc.gpsimd.indirect_copy(gout, tt, gi[:, : nv_per_group[qi] // 16], True)
            gsc = gscr.tile([128, nv_per_group[qi]], f32, tag="gsc")
            nc.vector.tensor_scalar(
                out=gsc,
                in0=gout,
                scalar1=1.0,
                scalar2=None,
                op0=mybir.AluOpType.mult,
                op1=mybir.AluOpType.add,
                accum_out=finalrhs[:, 160 + qi : 161 + qi],
            )

        # ---------------- the two chains, interleaved ----------------
        # round r: forward step t=r+1 (up to MID), backward step t'=S-1-r
        # (down to MID+1).  Backward: y = A_t' * v_t' ; r = E^T-contract(y)
        # accumulated with the inject outer product -> v_{t'-1}.
        nrounds = max(MID, S - 1 - MID)
        pre_queue = []
        for b0 in range(0, BC, GB):
            pre_queue.append((1, b0))
            pre_queue.append((2, b0))
        stt_queue = [(c, b) for c in (0, 3, 1, 2) for b in range(BC)]
        for r in range(nrounds):
            # interleave remaining preprocessing (chunks 1 and 2)
            if r % 8 == 1 and pre_queue:
                emit_pre_group(*pre_queue.pop(0))
            # spread the numerator's emission gathers across the rounds
            if r % 2 == 0 and stt_queue:
                emit_gather_b(*stt_queue.pop(0))
            if r in (40, 90, 140, 190):
                emit_quarter_gather((r - 40) // 50)
            tb = S - 1 - r
            if tb >= MID + 1:
                cb, tlb = divmod(tb, T)
                y = uvp.tile([T, BC], bf16, tag="y", name=f"y{tb}")
                nc.vector.tensor_tensor(
                    out=y, in0=v_psum, in1=a_ch[cb][:, tlb, :],
                    op=mybir.AluOpType.mult,
                )
                v_new = rpool.tile([T, BC], f32, tag="r")
                if tb - 1 in inj_rounds:
                    nc.tensor.matmul(v_new, et_sb, y, start=True, stop=False)
                    nc.tensor.matmul(
                        v_new, expendr, d_sb[:, tb - 1 - MID, :],
                        start=False, stop=True,
                    )
                else:
                    nc.tensor.matmul(v_new, et_sb, y, start=True, stop=True)
                v_psum = v_new
            tf = r + 1
            if tf <= MID:
                cf, tlf = divmod(tf, T)
                q = qpool.tile([T, BC], f32, tag="q")
                nc.tensor.matmul(q, e_sb, u_prev, start=True, stop=True)
                u_cur = uvp.tile([T, BC], bf16, tag="u", name=f"u{tf}")
                nc.vector.tensor_tensor(
                    out=u_cur, in0=q, in1=a_ch[cf][:, tlf, :],
                    op=mybir.AluOpType.mult,
                )
                u_prev = u_cur

        # ---------------- combine + final reduce ----------------
        # z_b = sum_i u_MID[i,b] * v_MID[i,b]
        nc.vector.tensor_tensor(
            out=finalrhs[:, 128:160], in0=v_psum, in1=u_prev,
            op=mybir.AluOpType.mult,
        )

        op = opool.tile([1, 168], f32)
        nc.tensor.matmul(op, ones, finalrhs, start=True, stop=True)
        outsb = consts.tile([1, 168], f32)
        nc.vector.tensor_copy(outsb, op)
        nc.sync.dma_start(outv[:, :], outsb)

    nc.compile()
    return nc


def _host_prep(logits, label, mask):
    """Per-core input marshalling (numpy only)."""
    logits = np.ascontiguousarray(np.asarray(logits, dtype=np.float32))
    label = np.asarray(label).astype(np.int32)
    mask = np.asarray(mask).astype(bool)
    lengths = mask.sum(axis=1).astype(np.int64)
    assert lengths.min() >= MID + 1, "meet-in-the-middle needs len >= MID+1"
    need_mask = not mask.all()
    ND = S - MID

    in_maps = []
    meta = []
    all_qlists = []  # per core: per quarter: index list
    for c in range(NCORES):
        lo, hi = c * BC, (c + 1) * BC
        lg = logits[lo:hi]
        lb = label[lo:hi]
        mk = mask[lo:hi]
        ln = lengths[lo:hi]

        # labels masked out of range -> one-hot never fires
        lbm = np.where(mk, lb, T).astype(np.float32)  # [BC, S]
        labf = np.empty((T, NCH, BC), np.float32)
        for ch in range(NCH):
            labf[:, ch, :] = lbm[:, ch * T:(ch + 1) * T].T

        # backward-half log-space mask (chunks 2 and 3), [s, c-2, b]:
        # 0 where alive, -60000 where dead so exp(em + m - c0) == 0.
        maskt = np.empty((T, 2, BC), np.float32)
        for ch in (2, 3):
            maskt[:, ch - 2, :] = np.where(
                mk[:, ch * T:(ch + 1) * T].T, 0.0, -60000.0
            )

        # inject indicator rows: dmat[0, k, b] = [len_b - 1 == MID + k]
        import ml_dtypes
        dm = np.zeros((1, ND, BC), ml_dtypes.bfloat16)
        dm[0, ln - 1 - MID, np.arange(BC)] = 1.0

        # transition gathers: for each (b, s>=1): value trans[l_{s-1}, l_s]
        # masked -> zero slot; plus start/end gathers per b.
        qlists = [[] for _ in range(NQ)]
        lprev = lb[:, :-1]
        lcur = lb[:, 1:]
        mks = mk[:, 1:]
        for b in range(BC):
            qp = lprev[b][mks[b]] // 32
            off = (lprev[b][mks[b]] % 32) * T + lcur[b][mks[b]]
            for qq, oo in zip(qp, off):
                qlists[qq].append(oo)
            qlists[0].append(4097 + lb[b, 0])          # start_transitions[l0]
            qlists[0].append(4225 + lb[b, ln[b] - 1])  # end_transitions[l_last]
        all_qlists.append(qlists)
        meta.append((ln, lo, hi))
        in_maps.append(dict(lg=lg, labf=labf, maskt=maskt, dmat=dm))

    # shared num_valid per quarter across cores (pad with zero slot 4096);
    # each indirect_copy group of 16 partitions gathers nv indices, so nv is
    # the max PER-GROUP count (lists are split 8 ways round robin).
    nv = []
    for qi in range(NQ):
        m = max((len(q[qi]) + 7) // 8 for q in all_qlists)
        m = ((max(m, 16) + 15) // 16) * 16
        nv.append(m)

    for c in range(NCORES):
        gix = np.zeros((NQ, 128, max(nv) // 16), np.uint16)
        for qi in range(NQ):
            lst = all_qlists[c][qi]
            groups = [lst[g::8] for g in range(8)]
            for g in range(8):
                gg = groups[g] + [4096] * (nv[qi] - len(groups[g]))
                for i, v in enumerate(gg[: nv[qi]]):
                    sw, p = divmod(i, 16)
                    gix[qi, 16 * g + p, sw] = v
        in_maps[c]["gidx"] = gix
    inj_rounds = set((lengths - 1).tolist()) - {S - 1}
    return in_maps, meta, nv, need_mask, inj_rounds


def _host_prep_shared(transitions, start_transitions, end_transitions):
    trans = np.asarray(transitions, dtype=np.float32)
    startT = np.asarray(start_transitions, dtype=np.float32)
    endT = np.asarray(end_transitions, dtype=np.float32)
    ttabs = np.zeros((NQ, QT), np.float32)
    for qi in range(NQ):
        ttabs[qi, :4096] = trans[32 * qi:32 * (qi + 1), :].reshape(-1)
        ttabs[qi, 4096] = 0.0
        ttabs[qi, 4097:4225] = startT
        ttabs[qi, 4225:4353] = endT
    return (
        trans,
        startT.reshape(T, 1).copy(),
        endT.reshape(1, T).copy(),
        ttabs,
    )


LAST_RUN_INFO = {}


def kernel(
    logits,
    label,
    mask,
    transitions,
    start_transitions,
    end_transitions,
    _trace=False,
    _tmpdir=None,
):
    from concourse.bass_utils import run_bass_kernel_spmd

    in_maps, meta, nv, need_mask, inj_rounds = _host_prep(logits, label, mask)
    trans, startv, endr, ttabs = _host_prep_shared(
        transitions, start_transitions, end_transitions
    )
    for m in in_maps:
        m["transm"] = trans
        m["startv"] = startv
        m["endr"] = endr
        m["ttabs"] = ttabs

    nc = _build_program(nv, need_mask, inj_rounds)
    kwargs = {}
    if _trace:
        kwargs = dict(trace=True, tmpdir=_tmpdir)
    res = run_bass_kernel_spmd(nc, in_maps, core_ids=list(range(NCORES)), **kwargs)
    LAST_RUN_INFO["exec_time_ns"] = res.exec_time_ns
    LAST_RUN_INFO["profile_json"] = res.profile_json

    total_score = 0.0
    total_denom = 0.0
    for c in range(NCORES):
        out = np.asarray(res.results[c]["outv"], np.float64).reshape(-1)
        ln = meta[c][0].astype(np.float64)
        em_sum = out[0:128].sum()
        z = out[128:160]
        tq = out[160:164].sum() / 16.0
        denom = np.log(z) + ln * C0
        total_score += em_sum + tq
        total_denom += denom.sum()
    loss = -(total_score - total_denom) / B
    return np.asarray(loss, dtype=np.float32)

